# revision 12
# baseline (speedup 1.0000x reference)
"""Trainium2 Bass kernel for a dense transformer block (pre-LN, causal MHA + FFN).

Reference computation (B=256, T=256, C=384, H=6, hd=64, D_FF=1536):
    h  = LN(x; g1, b1) ; q,k,v = per-head h @ W{q,k,v}
    wei = softmax(causal(q @ k^T * sqrt(C)))
    sa  = concat_heads(wei @ v) @ w_proj + b_proj ; x = x + sa
    h2  = LN(x; g2, b2) ; out = x + relu(h2 @ w1 + b1) @ w2 + b2

Sharding: pure data-parallel over batch B across 8 NeuronCores (32 seqs/core).
Weights replicated, LN gains/biases and the sqrt(C) scale folded host-side.

Design notes (v2):
- All transposes (h, q, k, wei, att, h2) run on the DMA crossbar
  (dma_start_transpose, 2-byte dtype) issued from the SP HWDGE queues - the
  PE runs only real matmuls, the DVE no longer does PSUM->SBUF transpose
  copies.
- q/k are computed token-major like v (sharing the h^T stationary, N=384
  moving) and DMA-transposed into head-major qT/kT; odd heads land on
  partitions 64-127 and are relocated to base-0 tiles by a small SBUF->SBUF
  DMA so every S matmul uses base partition 0.
- Softmax: one DVE tensor_tensor_reduce per S block computes
  Sneg = -(S + causal_mask) and negmax = min(Sneg) = -rowmax in a single op;
  ACT exp reads Sneg with scale=-1, bias=negmax. Row sums come for free from
  the att matmul via a ones-column appended to v (N=65); normalization is one
  reciprocal + one broadcast multiply per token tile.
- rstd for LN = (var + eps) ** -0.5 in one DVE tensor_scalar (add/pow) - the
  ACT engine runs Exp only (no activation-table thrash).
- relu and residual adds run on gpsimd (Pool); x/out/weight DMAs on SP.
- The per-sequence stages are software-pipelined: body i runs attention+proj+
  FFN for seq i-1, S for seq i, and LN1+QKV for seq i+1, so the PE always has
  matmul work while softmax/LN latency drains on DVE/ACT/DMA.
"""

import sys

for _p in ("/opt/trn_rl_repo", "/opt/pypackages"):
    if _p not in sys.path:
        sys.path.append(_p)

import numpy as np
import ml_dtypes

import concourse.bass as bass
import concourse.mybir as mybir
import concourse.tile as tile
from concourse.bass_utils import run_bass_kernel_spmd

# Problem constants (hardcoded per harness contract).
B, T, C = 256, 256, 384
H, HD = 6, 64
DFF = 4 * C  # 1536
SCALE = float(C) ** 0.5
LN_EPS = 1e-5
N_CORES = 8
B_SH = B // N_CORES          # 32 seqs per core
TOK = B_SH * T               # 8192 tokens per core
P = 128                      # partitions
NT = TOK // P                # token tiles per core
CCH = C // P                 # 3 contraction chunks of 128
NFF = DFF // P               # 12 ff groups

F32 = mybir.dt.float32
BF16 = mybir.dt.bfloat16

_BF = ml_dtypes.bfloat16

_CACHE = {}


def _hoist_extra_waits(nc):
    """This container's walrus supports one sync-wait per instruction; Tile
    attaches several. Hoist all-but-one onto NoOps on the same engine just
    before the instruction (engine-order preserving, deadlock-free since
    every sem's producer precedes the consumer in Tile's global schedule)."""
    for f in nc.m.functions:
        for blk in f.blocks:
            new_insts, dirty = [], False
            for ins in blk.instructions:
                si = ins.sync_info
                waits = list(si.on_wait) if (si is not None and si.on_wait) else []
                if len(waits) > 1:
                    for w in waits[:-1]:
                        nop = mybir.InstNoOp(name=f"wsplit_{nc.next_id()}")
                        nop.engine = ins.engine
                        nop.sync_info = mybir.SyncInfo(on_wait=[w], on_update=[])
                        nc.inst_map[nop.name] = nop
                        new_insts.append(nop)
                    ins.sync_info = mybir.SyncInfo(
                        on_wait=[waits[-1]],
                        on_update=list(si.on_update) if si.on_update else [],
                    )
                    dirty = True
                new_insts.append(ins)
            if dirty:
                blk.instructions = new_insts


def _build(has_bv, has_bp, has_b2, has_b1, has_bqk):
    nc = bass.Bass()

    x_h = nc.declare_dram_parameter("x", [TOK, C], F32, isOutput=False)
    wq_h = nc.declare_dram_parameter("wq_m", [C, C], BF16, isOutput=False)
    wk_h = nc.declare_dram_parameter("wk_m", [C, C], BF16, isOutput=False)
    wv_h = nc.declare_dram_parameter("wv_m", [C, C], BF16, isOutput=False)
    wp_h = nc.declare_dram_parameter("wp_m", [C, C], BF16, isOutput=False)
    w1_h = nc.declare_dram_parameter("w1_m", [C, DFF], BF16, isOutput=False)
    w2_h = nc.declare_dram_parameter("w2_m", [DFF, C], BF16, isOutput=False)
    bext_h = nc.declare_dram_parameter("bext_v", [5, C], BF16, isOutput=False)
    b1_h = nc.declare_dram_parameter("b1_v", [DFF], F32, isOutput=False)
    maskf_h = nc.declare_dram_parameter("maskf_m", [P, CCH * P], F32, isOutput=False)
    out_h = nc.declare_dram_parameter("out", [TOK, C], F32, isOutput=True)

    OP = mybir.AluOpType
    AF = mybir.ActivationFunctionType
    AX = mybir.AxisListType

    with tile.TileContext(nc) as tc:
        with (
            tc.tile_pool(name="const", bufs=1) as cst,
            tc.tile_pool(name="xs", bufs=5) as xp,
            tc.tile_pool(name="acts", bufs=2) as ap,
            tc.tile_pool(name="qkv", bufs=2) as qkvp,
            tc.tile_pool(name="attn", bufs=4) as atp,
            tc.tile_pool(name="wts", bufs=12) as wtp,
            tc.tile_pool(name="stats", bufs=8) as stp,
            tc.tile_pool(name="ffn", bufs=2) as ffp,
            tc.tile_pool(name="outs", bufs=3) as op_,
            tc.tile_pool(name="ps_s", bufs=2, space="PSUM") as pss,
            tc.tile_pool(name="ps_att", bufs=1, space="PSUM") as psa,
            tc.tile_pool(name="ps_big", bufs=3, space="PSUM") as psb,
            tc.tile_pool(name="ps_ff1", bufs=2, space="PSUM") as psf,
        ):
            # ---- constants / weights (resident) ----
            wq_sb = cst.tile([P, CCH, C], BF16)
            nc.sync.dma_start(out=wq_sb, in_=wq_h[:].rearrange("(o p) f -> p o f", p=P))
            wk_sb = cst.tile([P, CCH, C], BF16)
            nc.sync.dma_start(out=wk_sb, in_=wk_h[:].rearrange("(o p) f -> p o f", p=P))
            wv_sb = cst.tile([P, CCH, C], BF16)
            nc.sync.dma_start(out=wv_sb, in_=wv_h[:].rearrange("(o p) f -> p o f", p=P))
            wp_sb = cst.tile([P, CCH, C], BF16)
            nc.sync.dma_start(out=wp_sb, in_=wp_h[:].rearrange("(o p) f -> p o f", p=P))
            w1_sb = cst.tile([P, CCH, DFF], BF16)
            nc.sync.dma_start(out=w1_sb, in_=w1_h[:].rearrange("(o p) f -> p o f", p=P))
            w2_sb = cst.tile([P, NFF, C], BF16)
            nc.sync.dma_start(out=w2_sb, in_=w2_h[:].rearrange("(o p) f -> p o f", p=P))
            b1_sb = cst.tile([P, NFF], F32)
            nc.sync.dma_start(out=b1_sb, in_=b1_h[:].rearrange("(o p) -> p o", p=P))
            maskf_sb = cst.tile([P, CCH * P], F32)
            nc.sync.dma_start(out=maskf_sb, in_=maskf_h[:])
            eps_sb = cst.tile([P, 1], F32)
            nc.vector.memset(eps_sb, LN_EPS)
            # ones row + free-dim biases for the broadcast-bias matmul trick
            ones_sb = cst.tile([1, P], BF16)
            nc.vector.memset(ones_sb, 1.0)
            bext_sb = cst.tile([1, 5, C], BF16)
            nc.sync.dma_start(
                out=bext_sb, in_=bext_h[:].rearrange("o (u f) -> u o f", u=1)
            )

            def layer_norm(x_in, h_out, tag):
                """h_out (bf16) = (x_in - mean) * (var + eps) ** -0.5; LN gains
                and biases fold into downstream weights. All on DVE."""
                mv = stp.tile([P, 6], F32, tag="bnstats", name=f"mv_{tag}")
                nc.vector.bn_stats(out=mv, in_=x_in)
                agg = stp.tile([P, 2], F32, tag="bnagg", name=f"agg_{tag}")
                nc.vector.bn_aggr(out=agg, in_=mv)
                rstd = stp.tile([P, 1], F32, tag="rstd", name=f"rstd_{tag}")
                # rstd = exp(-0.5 * ln(var + eps)): Ln and Exp share one ACT
                # table set (natural_log_exp_and_others), so no table thrash
                # with the softmax Exp (Sqrt would force a set switch).
                nc.scalar.activation(
                    out=rstd, in_=agg[:, 1:2], func=AF.Ln, bias=eps_sb, scale=1.0
                )
                nc.scalar.activation(
                    out=rstd, in_=rstd, func=AF.Exp, scale=-0.5
                )
                # apply on Pool (SBUF->SBUF; Pool cannot touch PSUM)
                nc.gpsimd.tensor_scalar(
                    out=h_out, in0=x_in,
                    scalar1=agg[:, 0:1], scalar2=rstd,
                    op0=OP.subtract, op1=OP.mult,
                )

            def bcast(ap_obj, n):
                """Append a stride-0 free dim of size n (free-dim broadcast)."""
                return bass.AP(
                    tensor=ap_obj.tensor, offset=ap_obj.offset,
                    ap=[*ap_obj.ap, [0, n]],
                )

            # per-seq pipeline state
            st = [dict() for _ in range(B_SH)]

            def ph_x_dma(s):
                d = st[s]
                d["x"] = []
                for j in range(2):
                    it = 2 * s + j
                    x_i = xp.tile([P, C], F32, tag="x", bufs=10, name=f"x{j}_{s}")
                    nc.sync.dma_start(out=x_i, in_=x_h[it * P:(it + 1) * P, :])
                    d["x"].append(x_i)

            def ph_ln1(s, j):
                d = st[s]
                if j == 0:
                    d["hT"] = ap.tile([P, CCH, T], BF16, tag="hT", name=f"hT_{s}")
                h_i = ap.tile([P, C], BF16, tag="h", name=f"h{j}_{s}")
                layer_norm(d["x"][j], h_i, f"l1_{j}_{s}")
                nc.sync.dma_start_transpose(
                    out=d["hT"][:, :, j * P:(j + 1) * P], in_=h_i
                )

            def ph_vqk(s):
                d = st[s]
                hT = d["hT"]
                v_aug = qkvp.tile([P, 2, H, HD + 1], BF16, tag="vaug",
                                  bufs=3, name=f"vaug_{s}")
                nc.gpsimd.memset(v_aug[:, :, :, HD:HD + 1], 1.0)
                d["v"] = v_aug
                qT = qkvp.tile([P, CCH, T], BF16, tag="qT", name=f"qT_{s}")
                kT = qkvp.tile([P, CCH, T], BF16, tag="kT", name=f"kT_{s}")
                for j in range(2):
                    hT_j = hT[:, :, j * P:(j + 1) * P]
                    for (w_sb, bi, nm) in ((wv_sb, 0, "v"), (wq_sb, 3, "q"),
                                           (wk_sb, 4, "k")):
                        has_b = has_bv if nm == "v" else has_bqk
                        ps = psb.tile([P, C], F32, tag="big", name=f"ps{nm}{j}_{s}")
                        for c in range(CCH):
                            nc.tensor.matmul(
                                ps, hT_j[:, c, :], w_sb[:, c, :],
                                start=(c == 0),
                                stop=(c == CCH - 1 and not has_b),
                            )
                        if has_b:
                            nc.tensor.matmul(ps, ones_sb, bext_sb[:, bi, :],
                                             start=False, stop=True)
                        if nm == "v":
                            nc.vector.tensor_copy(
                                out=v_aug[:, j, :, 0:HD], in_=ps[:].rearrange(
                                    "p (h d) -> p h d", h=H),
                            )
                        else:
                            sb = ap.tile([P, C], BF16, tag="qk",
                                         bufs=4, name=f"{nm}sb{j}_{s}")
                            nc.vector.tensor_copy(out=sb, in_=ps)
                            nc.sync.dma_start_transpose(
                                out=(qT if nm == "q" else kT)[
                                    :, :, j * P:(j + 1) * P],
                                in_=sb,
                            )
                # odd heads (partitions 64-127) -> base-0 tiles
                qTo = qkvp.tile([P, CCH, T], BF16, tag="qTo", name=f"qTo_{s}")
                kTo = qkvp.tile([P, CCH, T], BF16, tag="kTo", name=f"kTo_{s}")
                nc.sync.dma_start(out=qTo[0:HD, :, :], in_=qT[HD:P, :, :])
                nc.sync.dma_start(out=kTo[0:HD, :, :], in_=kT[HD:P, :, :])
                d["qT"], d["kT"], d["qTo"], d["kTo"] = qT, kT, qTo, kTo

            def ph_S(s, heads):
                d = st[s]
                if "wT" not in d:
                    d["wT"] = [None] * H
                for hh in heads:
                    g = hh // 2
                    q_t = (d["qT"] if hh % 2 == 0 else d["qTo"])[0:HD, g, :]
                    k_t = (d["kT"] if hh % 2 == 0 else d["kTo"])[0:HD, g, :]
                    psS = pss.tile([P, CCH, P], F32, tag="s", name=f"s{hh}_{s}")
                    nc.tensor.matmul(psS[:, 0, :], q_t[:, 0:P], k_t[:, 0:P],
                                     start=True, stop=True)
                    nc.tensor.matmul(psS[:, 1:3, :], q_t[:, P:T], k_t[:, 0:T],
                                     start=True, stop=True)
                    smk = atp.tile([P, CCH, P], F32, tag="smk", name=f"sm{hh}_{s}")
                    nmax = stp.tile([P, 2], F32, tag="nmax", name=f"nm{hh}_{s}")
                    nc.vector.tensor_tensor(
                        out=smk, in0=psS, in1=maskf_sb[:].rearrange(
                            "p (u f) -> p u f", u=CCH),
                        op=OP.add,
                    )
                    nc.vector.tensor_reduce(
                        out=nmax[:, 0:1], in_=smk[:, 0, :], axis=AX.X,
                        op=OP.max, negate=True,
                    )
                    nc.vector.tensor_reduce(
                        out=nmax[:, 1:2], in_=smk[:, 1:3, :], axis=AX.XY,
                        op=OP.max, negate=True,
                    )
                    wei = atp.tile([P, CCH, P], BF16, tag="wei", name=f"we{hh}_{s}")
                    nc.scalar.activation(
                        out=wei[:, 0, :], in_=smk[:, 0, :], func=AF.Exp,
                        bias=nmax[:, 0:1], scale=1.0,
                    )
                    nc.scalar.activation(
                        out=wei[:, 1:3, :], in_=smk[:, 1:3, :], func=AF.Exp,
                        bias=nmax[:, 1:2], scale=1.0,
                    )
                    wT = wtp.tile([P, CCH, P], BF16, tag="wT", name=f"wT{hh}_{s}")
                    nc.sync.dma_start_transpose(out=wT[:, 0:1, :], in_=wei[:, 0, :])
                    nc.sync.dma_start_transpose(
                        out=wT[:, 1:3, :],
                        in_=wei[:, 1:3, :].rearrange("p u f -> p (u f)"),
                    )
                    d["wT"][hh] = wT

            def ph_att(s, j):
                d = st[s]
                ps_att = psa.tile([P, H, HD + 1], F32, tag="att",
                                  name=f"psatt{j}_{s}")
                for hh in range(H):
                    wT = d["wT"][hh]
                    if j == 0:
                        nc.tensor.matmul(
                            ps_att[:, hh, :], wT[:, 0, :], d["v"][:, 0, hh, :],
                            start=True, stop=True,
                        )
                    else:
                        for cs in range(2):
                            nc.tensor.matmul(
                                ps_att[:, hh, :], wT[:, 1 + cs, :],
                                d["v"][:, cs, hh, :],
                                start=(cs == 0), stop=(cs == 1),
                            )
                rs = stp.tile([P, H], F32, tag="rs", name=f"rs{j}_{s}")
                nc.vector.reciprocal(out=rs, in_=ps_att[:, :, HD:HD + 1])
                att_sb = atp.tile([P, C], BF16, tag="attsb", bufs=2,
                                  name=f"attsb{j}_{s}")
                nc.vector.tensor_tensor(
                    out=att_sb[:].rearrange("p (h d) -> p h d", h=H),
                    in0=ps_att[:, :, 0:HD], in1=bcast(rs, HD), op=OP.mult,
                )
                attT = atp.tile([P, CCH, P], BF16, tag="attT", bufs=2,
                                name=f"attT{j}_{s}")
                nc.sync.dma_start_transpose(out=attT, in_=att_sb)
                d[f"attT{j}"] = attT

            def ph_proj(s, j):
                d = st[s]
                attT = d[f"attT{j}"]
                ps_sa = psb.tile([P, C], F32, tag="big", name=f"ps_sa{j}_{s}")
                for c in range(CCH):
                    nc.tensor.matmul(
                        ps_sa, attT[:, c, :], wp_sb[:, c, :],
                        start=(c == 0), stop=(c == CCH - 1 and not has_bp),
                    )
                if has_bp:
                    nc.tensor.matmul(ps_sa, ones_sb, bext_sb[:, 1, :],
                                     start=False, stop=True)
                x2_i = xp.tile([P, C], F32, tag="x2", bufs=2, name=f"x2{j}_{s}")
                nc.vector.tensor_tensor(out=x2_i, in0=ps_sa, in1=d["x"][j],
                                        op=OP.add)
                d.setdefault("x2", {})[j] = x2_i
                if "h2T" not in d:
                    d["h2T"] = ap.tile([P, CCH, T], BF16, tag="h2T",
                                       name=f"h2T_{s}")
                h2_i = ap.tile([P, C], BF16, tag="h2", name=f"h2{j}_{s}")
                layer_norm(x2_i, h2_i, f"l2_{j}_{s}")
                nc.sync.dma_start_transpose(
                    out=d["h2T"][:, :, j * P:(j + 1) * P], in_=h2_i
                )

            def ph_ffn1(s):
                d = st[s]
                aT = ffp.tile([P, NFF, T], BF16, tag="aT", name=f"aT_{s}")
                d["aT"] = aT
                for g2 in range(NFF // 2):
                    ps_a = psf.tile([P, 2, T], F32, tag="ff1", name=f"psa{g2}_{s}")
                    for i in range(2):
                        g = 2 * g2 + i
                        for c in range(CCH):
                            nc.tensor.matmul(
                                ps_a[:, i, :],
                                w1_sb[:, c, g * P:(g + 1) * P], d["h2T"][:, c, :],
                                start=(c == 0), stop=(c == CCH - 1),
                            )
                    if has_b1:
                        for i in range(2):
                            g = 2 * g2 + i
                            nc.scalar.activation(
                                out=aT[:, g, :], in_=ps_a[:, i, :],
                                func=AF.Relu, bias=b1_sb[:, g:g + 1], scale=1.0,
                            )
                    else:
                        nc.scalar.activation(
                            out=aT[:, 2 * g2:2 * g2 + 2, :], in_=ps_a,
                            func=AF.Relu,
                        )

            def ph_ffn2(s, j):
                d = st[s]
                ps_y = psb.tile([P, C], F32, tag="big", name=f"ps_y{j}_{s}")
                for g in range(NFF):
                    nc.tensor.matmul(
                        ps_y, d["aT"][:, g, j * P:(j + 1) * P], w2_sb[:, g, :],
                        start=(g == 0), stop=(g == NFF - 1 and not has_b2),
                    )
                if has_b2:
                    nc.tensor.matmul(ps_y, ones_sb, bext_sb[:, 2, :],
                                     start=False, stop=True)
                o_i = op_.tile([P, C], F32, tag="o", name=f"o{j}_{s}")
                nc.vector.tensor_tensor(out=o_i, in0=ps_y, in1=d["x2"][j],
                                        op=OP.add)
                it = 2 * s + j
                nc.sync.dma_start(out=out_h[it * P:(it + 1) * P, :], in_=o_i)
                if j == 1:
                    st[s] = {}  # drop refs

            # ---- software-pipelined schedule ----
            ph_x_dma(0)
            if B_SH > 1:
                ph_x_dma(1)
            ph_ln1(0, 0)
            ph_ln1(0, 1)
            ph_vqk(0)
            for i in range(B_SH + 1):
                a = i - 1  # attention/proj/ffn seq
                nxt = i + 1 < B_SH
                if nxt:
                    ph_ln1(i + 1, 0)
                if 0 <= a:
                    ph_att(a, 0)
                if i < B_SH:
                    ph_S(i, [0, 2])
                if nxt:
                    ph_ln1(i + 1, 1)
                if 0 <= a:
                    ph_att(a, 1)
                if i < B_SH:
                    ph_S(i, [4, 1])
                if 0 <= a:
                    ph_proj(a, 0)
                if i < B_SH:
                    ph_S(i, [3, 5])
                if 0 <= a:
                    ph_proj(a, 1)
                if i + 2 < B_SH:
                    ph_x_dma(i + 2)
                if nxt:
                    ph_vqk(i + 1)
                if 0 <= a:
                    ph_ffn1(a)
                    ph_ffn2(a, 0)
                    ph_ffn2(a, 1)

    _hoist_extra_waits(nc)
    return nc


def _prep_weights(inputs):
    f32 = np.float32
    g1 = inputs["ln1_g"].astype(f32)
    b1l = inputs["ln1_b"].astype(f32)
    g2 = inputs["ln2_g"].astype(f32)
    b2l = inputs["ln2_b"].astype(f32)
    wq, wk, wv = (inputs[k].astype(f32) for k in ("wq", "wk", "wv"))
    w1 = inputs["w1"].astype(f32)

    # fold LN gains/biases + attention scale
    wq_f = wq * g1[None, :, None] * SCALE          # [H, C, hd]
    bq = SCALE * np.einsum("c,hcd->hd", b1l, wq)   # [H, hd]
    wk_f = wk * g1[None, :, None]
    bk = np.einsum("c,hcd->hd", b1l, wk)
    wv_f = wv * g1[None, :, None]
    bv = np.einsum("c,hcd->hd", b1l, wv)
    w1_f = w1 * g2[:, None]
    b1f = inputs["b1"].astype(f32) + b2l @ w1

    # head-major column layout [C, H*hd]
    to_mat = lambda w: np.ascontiguousarray(w.transpose(1, 0, 2).reshape(C, C))
    tri = np.triu(np.full((P, P), -1e9, dtype=f32), k=1)
    d = {
        "wq_m": to_mat(wq_f).astype(_BF),
        "wk_m": to_mat(wk_f).astype(_BF),
        "wv_m": to_mat(wv_f).astype(_BF),
        "wp_m": np.ascontiguousarray(inputs["w_proj"].astype(f32)).astype(_BF),
        "w1_m": np.ascontiguousarray(w1_f).astype(_BF),
        "w2_m": np.ascontiguousarray(inputs["w2"].astype(f32)).astype(_BF),
        "b1_v": np.ascontiguousarray(b1f).astype(f32),
        "maskf_m": np.concatenate([tri, np.zeros((P, P), f32), tri], axis=1),
    }
    bv_r = bv.reshape(C)
    bp_r = inputs["b_proj"].astype(f32)
    b2_r = inputs["b2"].astype(f32)
    bq_r = bq.reshape(C)
    bk_r = bk.reshape(C)
    d["bext_v"] = np.stack([bv_r, bp_r, b2_r, bq_r, bk_r]).astype(_BF)
    flags = (bool(np.any(bv_r)), bool(np.any(bp_r)), bool(np.any(b2_r)),
             bool(np.any(b1f)),
             bool(np.any(bq_r)) or bool(np.any(bk_r)))
    return d, flags


def kernel(**inputs) -> np.ndarray:
    x = np.ascontiguousarray(inputs["x"].astype(np.float32))
    weights, flags = _prep_weights(inputs)

    if flags not in _CACHE:
        _CACHE[flags] = _build(*flags)
    nc = _CACHE[flags]

    xs = x.reshape(N_CORES, TOK, C)
    in_maps = [dict(weights, x=np.ascontiguousarray(xs[i])) for i in range(N_CORES)]
    import os

    kwargs = {}
    if os.environ.get("BASS_PROF"):
        kwargs = {"trace": True, "trace_cores": [0]}
    res = run_bass_kernel_spmd(nc, in_maps, list(range(N_CORES)), **kwargs)
    globals()["LAST_RESULTS"] = res
    out = np.stack([res.results[i]["out"] for i in range(N_CORES)])
    return out.reshape(B, T, C).astype(np.float32)


# revision 14
# speedup vs baseline: 1.9135x; 1.9135x over previous
"""Trainium2 Bass kernel for a dense transformer block (pre-LN, causal MHA + FFN).

Reference computation (B=256, T=256, C=384, H=6, hd=64, D_FF=1536):
    h  = LN(x; g1, b1) ; q,k,v = per-head h @ W{q,k,v}
    wei = softmax(causal(q @ k^T * sqrt(C)))
    sa  = concat_heads(wei @ v) @ w_proj + b_proj ; x = x + sa
    h2  = LN(x; g2, b2) ; out = x + relu(h2 @ w1 + b1) @ w2 + b2

Sharding: pure data-parallel over batch B across 8 NeuronCores (32 seqs/core).
Weights replicated, LN gains/biases and the sqrt(C) scale folded host-side.

Design notes (v2):
- All transposes (h, q, k, wei, att, h2) run on the DMA crossbar
  (dma_start_transpose, 2-byte dtype) issued from the SP HWDGE queues - the
  PE runs only real matmuls, the DVE no longer does PSUM->SBUF transpose
  copies.
- q/k are computed token-major like v (sharing the h^T stationary, N=384
  moving) and DMA-transposed into head-major qT/kT; odd heads land on
  partitions 64-127 and are relocated to base-0 tiles by a small SBUF->SBUF
  DMA so every S matmul uses base partition 0.
- Softmax: one DVE tensor_tensor_reduce per S block computes
  Sneg = -(S + causal_mask) and negmax = min(Sneg) = -rowmax in a single op;
  ACT exp reads Sneg with scale=-1, bias=negmax. Row sums come for free from
  the att matmul via a ones-column appended to v (N=65); normalization is one
  reciprocal + one broadcast multiply per token tile.
- rstd for LN = (var + eps) ** -0.5 in one DVE tensor_scalar (add/pow) - the
  ACT engine runs Exp only (no activation-table thrash).
- relu and residual adds run on gpsimd (Pool); x/out/weight DMAs on SP.
- The per-sequence stages are software-pipelined: body i runs attention+proj+
  FFN for seq i-1, S for seq i, and LN1+QKV for seq i+1, so the PE always has
  matmul work while softmax/LN latency drains on DVE/ACT/DMA.
"""

import sys

for _p in ("/opt/trn_rl_repo", "/opt/pypackages"):
    if _p not in sys.path:
        sys.path.append(_p)

import numpy as np
import ml_dtypes

import concourse.bass as bass
import concourse.mybir as mybir
import concourse.tile as tile
from concourse.bass_utils import run_bass_kernel_spmd

# Problem constants (hardcoded per harness contract).
B, T, C = 256, 256, 384
H, HD = 6, 64
DFF = 4 * C  # 1536
SCALE = float(C) ** 0.5
LN_EPS = 1e-5
N_CORES = 8
B_SH = B // N_CORES          # 32 seqs per core
TOK = B_SH * T               # 8192 tokens per core
P = 128                      # partitions
NT = TOK // P                # token tiles per core
CCH = C // P                 # 3 contraction chunks of 128
NFF = DFF // P               # 12 ff groups

F32 = mybir.dt.float32
BF16 = mybir.dt.bfloat16

_BF = ml_dtypes.bfloat16

_CACHE = {}


def _hoist_extra_waits(nc):
    """This container's walrus supports one sync-wait per instruction; Tile
    attaches several. Hoist all-but-one onto NoOps on the same engine just
    before the instruction (engine-order preserving, deadlock-free since
    every sem's producer precedes the consumer in Tile's global schedule)."""
    for f in nc.m.functions:
        for blk in f.blocks:
            new_insts, dirty = [], False
            for ins in blk.instructions:
                si = ins.sync_info
                waits = list(si.on_wait) if (si is not None and si.on_wait) else []
                if len(waits) > 1:
                    for w in waits[:-1]:
                        nop = mybir.InstNoOp(name=f"wsplit_{nc.next_id()}")
                        nop.engine = ins.engine
                        nop.sync_info = mybir.SyncInfo(on_wait=[w], on_update=[])
                        nc.inst_map[nop.name] = nop
                        new_insts.append(nop)
                    ins.sync_info = mybir.SyncInfo(
                        on_wait=[waits[-1]],
                        on_update=list(si.on_update) if si.on_update else [],
                    )
                    dirty = True
                new_insts.append(ins)
            if dirty:
                blk.instructions = new_insts


def _build(has_bv, has_bp, has_b2, has_b1, has_bqk):
    nc = bass.Bass()

    x_h = nc.declare_dram_parameter("x", [TOK, C], F32, isOutput=False)
    wq_h = nc.declare_dram_parameter("wq_m", [C, C], BF16, isOutput=False)
    wk_h = nc.declare_dram_parameter("wk_m", [C, C], BF16, isOutput=False)
    wv_h = nc.declare_dram_parameter("wv_m", [C, C], BF16, isOutput=False)
    wp_h = nc.declare_dram_parameter("wp_m", [C, C], BF16, isOutput=False)
    w1_h = nc.declare_dram_parameter("w1_m", [C, DFF], BF16, isOutput=False)
    w2_h = nc.declare_dram_parameter("w2_m", [DFF, C], BF16, isOutput=False)
    bext_h = nc.declare_dram_parameter("bext_v", [5, C], BF16, isOutput=False)
    b1_h = nc.declare_dram_parameter("b1_v", [DFF], F32, isOutput=False)
    maskf_h = nc.declare_dram_parameter("maskf_m", [P, CCH * P], F32, isOutput=False)
    out_h = nc.declare_dram_parameter("out", [TOK, C], F32, isOutput=True)

    OP = mybir.AluOpType
    AF = mybir.ActivationFunctionType
    AX = mybir.AxisListType

    with tile.TileContext(nc) as tc:
        with (
            tc.tile_pool(name="const", bufs=1) as cst,
            tc.tile_pool(name="xs", bufs=5) as xp,
            tc.tile_pool(name="acts", bufs=2) as ap,
            tc.tile_pool(name="qkv", bufs=2) as qkvp,
            tc.tile_pool(name="attn", bufs=4) as atp,
            tc.tile_pool(name="wts", bufs=12) as wtp,
            tc.tile_pool(name="stats", bufs=8) as stp,
            tc.tile_pool(name="ffn", bufs=2) as ffp,
            tc.tile_pool(name="outs", bufs=3) as op_,
            tc.tile_pool(name="ps_s", bufs=2, space="PSUM") as pss,
            tc.tile_pool(name="ps_att", bufs=1, space="PSUM") as psa,
            tc.tile_pool(name="ps_big", bufs=3, space="PSUM") as psb,
            tc.tile_pool(name="ps_ff1", bufs=2, space="PSUM") as psf,
        ):
            # ---- constants / weights (resident) ----
            wq_sb = cst.tile([P, CCH, C], BF16)
            nc.scalar.dma_start(out=wq_sb, in_=wq_h[:].rearrange("(o p) f -> p o f", p=P))
            wk_sb = cst.tile([P, CCH, C], BF16)
            nc.scalar.dma_start(out=wk_sb, in_=wk_h[:].rearrange("(o p) f -> p o f", p=P))
            wv_sb = cst.tile([P, CCH, C], BF16)
            nc.scalar.dma_start(out=wv_sb, in_=wv_h[:].rearrange("(o p) f -> p o f", p=P))
            wp_sb = cst.tile([P, CCH, C], BF16)
            nc.scalar.dma_start(out=wp_sb, in_=wp_h[:].rearrange("(o p) f -> p o f", p=P))
            w1_sb = cst.tile([P, CCH, DFF], BF16)
            nc.scalar.dma_start(out=w1_sb, in_=w1_h[:].rearrange("(o p) f -> p o f", p=P))
            w2_sb = cst.tile([P, NFF, C], BF16)
            nc.scalar.dma_start(out=w2_sb, in_=w2_h[:].rearrange("(o p) f -> p o f", p=P))
            b1_sb = cst.tile([P, NFF], F32)
            nc.scalar.dma_start(out=b1_sb, in_=b1_h[:].rearrange("(o p) -> p o", p=P))
            maskf_sb = cst.tile([P, CCH * P], F32)
            nc.scalar.dma_start(out=maskf_sb, in_=maskf_h[:])
            eps_sb = cst.tile([P, 1], F32)
            nc.vector.memset(eps_sb, LN_EPS)
            # ones row + free-dim biases for the broadcast-bias matmul trick
            ones_sb = cst.tile([1, P], BF16)
            nc.vector.memset(ones_sb, 1.0)
            bext_sb = cst.tile([1, 5, C], BF16)
            nc.scalar.dma_start(
                out=bext_sb, in_=bext_h[:].rearrange("o (u f) -> u o f", u=1)
            )

            def layer_norm(x_in, h_out, tag):
                """h_out (bf16) = (x_in - mean) * (var + eps) ** -0.5; LN gains
                and biases fold into downstream weights. All on DVE."""
                mv = stp.tile([P, 6], F32, tag="bnstats", name=f"mv_{tag}")
                nc.vector.bn_stats(out=mv, in_=x_in)
                agg = stp.tile([P, 2], F32, tag="bnagg", name=f"agg_{tag}")
                nc.vector.bn_aggr(out=agg, in_=mv)
                rstd = stp.tile([P, 1], F32, tag="rstd", name=f"rstd_{tag}")
                # rstd = exp(-0.5 * ln(var + eps)): Ln and Exp share one ACT
                # table set (natural_log_exp_and_others), so no table thrash
                # with the softmax Exp (Sqrt would force a set switch).
                nc.scalar.activation(
                    out=rstd, in_=agg[:, 1:2], func=AF.Ln, bias=eps_sb, scale=1.0
                )
                nc.scalar.activation(
                    out=rstd, in_=rstd, func=AF.Exp, scale=-0.5
                )
                nmr = stp.tile([P, 1], F32, tag="nmr", name=f"nmr_{tag}")
                nc.vector.tensor_scalar(
                    out=nmr, in0=agg[:, 0:1],
                    scalar1=rstd, scalar2=-1.0,
                    op0=OP.mult, op1=OP.mult,
                )
                # apply on ACT: h = x*rstd + (-mean*rstd); Identity is in the
                # same table set as Exp/Ln, so still no table switches.
                nc.scalar.activation(
                    out=h_out, in_=x_in, func=AF.Identity,
                    bias=nmr, scale=rstd,
                )

            def bcast(ap_obj, n):
                """Append a stride-0 free dim of size n (free-dim broadcast)."""
                return bass.AP(
                    tensor=ap_obj.tensor, offset=ap_obj.offset,
                    ap=[*ap_obj.ap, [0, n]],
                )

            # per-seq pipeline state
            st = [dict() for _ in range(B_SH)]

            def ph_x_dma(s):
                d = st[s]
                x_i = xp.tile([P, 2, C], F32, tag="x", bufs=5, name=f"x_{s}")
                it = 2 * s
                nc.scalar.dma_start(
                    out=x_i,
                    in_=x_h[it * P:(it + 2) * P, :].rearrange(
                        "(j p) f -> p j f", p=P),
                )
                d["x"] = x_i

            def ph_ln1(s):
                d = st[s]
                h_t = ap.tile([P, 2, C], BF16, tag="h", name=f"h_{s}")
                for j in range(2):
                    layer_norm(d["x"][:, j, :], h_t[:, j, :], f"l1_{j}_{s}")
                hT = ap.tile([P, 2 * CCH, P], BF16, tag="hT", name=f"hT_{s}")
                nc.sync.dma_start_transpose(
                    out=hT, in_=h_t[:].rearrange("p j f -> p (j f)")
                )
                d["hT"] = hT

            def ph_v(s):
                d = st[s]
                v_aug = qkvp.tile([P, 2, H, HD + 1], BF16, tag="vaug",
                                  bufs=3, name=f"vaug_{s}")
                nc.gpsimd.memset(v_aug[:, :, :, HD:HD + 1], 1.0)
                d["v"] = v_aug
                for j in range(2):
                    ps = psb.tile([P, C], F32, tag="big", name=f"psv{j}_{s}")
                    for c in range(CCH):
                        nc.tensor.matmul(
                            ps, d["hT"][:, 3 * j + c, :], wv_sb[:, c, :],
                            start=(c == 0),
                            stop=(c == CCH - 1 and not has_bv),
                        )
                    if has_bv:
                        nc.tensor.matmul(ps, ones_sb, bext_sb[:, 0, :],
                                         start=False, stop=True)
                    nc.vector.tensor_copy(
                        out=v_aug[:, j, :, 0:HD],
                        in_=ps[:].rearrange("p (h e) -> p h e", h=H),
                    )

            def ph_qk(s):
                d = st[s]
                qk_sb = ap.tile([P, 2, 2, C], BF16, tag="qk", name=f"qk_{s}")
                for j in range(2):
                    for (w_sb, bi, qi) in ((wq_sb, 3, 0), (wk_sb, 4, 1)):
                        ps = psb.tile([P, C], F32, tag="big",
                                      name=f"psqk{j}{qi}_{s}")
                        for c in range(CCH):
                            nc.tensor.matmul(
                                ps, d["hT"][:, 3 * j + c, :], w_sb[:, c, :],
                                start=(c == 0),
                                stop=(c == CCH - 1 and not has_bqk),
                            )
                        if has_bqk:
                            nc.tensor.matmul(ps, ones_sb, bext_sb[:, bi, :],
                                             start=False, stop=True)
                        nc.vector.tensor_copy(out=qk_sb[:, j, qi, :], in_=ps)
                # one xbar transpose for q+k of both tiles:
                # slot b = 6j + 3qi + c  ->  qkT[p, b, t]
                qkT = qkvp.tile([P, 2 * 2 * CCH, P], BF16, tag="qkT",
                                name=f"qkT_{s}")
                nc.sync.dma_start_transpose(
                    out=qkT, in_=qk_sb[:].rearrange("p j q f -> p (j q f)")
                )
                qkTo = qkvp.tile([P, 2 * 2 * CCH, P], BF16, tag="qkTo",
                                 name=f"qkTo_{s}")
                nc.scalar.dma_start(out=qkTo[0:HD, :, :], in_=qkT[HD:P, :, :])
                d["qkT"], d["qkTo"] = qkT, qkTo

            def ph_S(s, pair):
                d = st[s]
                if "wT" not in d:
                    d["wT"] = [None] * CCH
                wei = atp.tile([P, 2, CCH, P], BF16, tag="wei",
                               name=f"we{pair}_{s}")
                for i in range(2):
                    hh = 2 * pair + i
                    src_t = d["qkT"] if i == 0 else d["qkTo"]
                    # slot layout: [j, qk, chunk] -> 6j + 3qk + c
                    q0 = src_t[0:HD, pair, :]            # q, tile0
                    q1 = src_t[0:HD, 6 + pair, :]        # q, tile1
                    kr = src_t[0:HD, :, :].rearrange(
                        "p (j q c) t -> p q c j t", j=2, q=2)
                    psS = pss.tile([P, CCH, P], F32, tag="s",
                                   name=f"s{hh}_{s}")
                    nc.tensor.matmul(psS[:, 0, :], q0, kr[:, 1, pair, 0, :],
                                     start=True, stop=True)
                    nc.tensor.matmul(psS[:, 1:3, :], q1, kr[:, 1, pair, :, :],
                                     start=True, stop=True)
                    smk = atp.tile([P, CCH, P], F32, tag="smk",
                                   name=f"sm{hh}_{s}")
                    nmax = stp.tile([P, 2], F32, tag="nmax", name=f"nm{hh}_{s}")
                    nc.vector.tensor_tensor(
                        out=smk, in0=psS, in1=maskf_sb[:].rearrange(
                            "p (u f) -> p u f", u=CCH),
                        op=OP.add,
                    )
                    nc.vector.tensor_reduce(
                        out=nmax[:, 0:1], in_=smk[:, 0, :], axis=AX.X,
                        op=OP.max, negate=True,
                    )
                    nc.vector.tensor_reduce(
                        out=nmax[:, 1:2], in_=smk[:, 1:3, :], axis=AX.XY,
                        op=OP.max, negate=True,
                    )
                    nc.scalar.activation(
                        out=wei[:, i, 0, :], in_=smk[:, 0, :], func=AF.Exp,
                        bias=nmax[:, 0:1], scale=1.0,
                    )
                    nc.scalar.activation(
                        out=wei[:, i, 1:3, :], in_=smk[:, 1:3, :], func=AF.Exp,
                        bias=nmax[:, 1:2], scale=1.0,
                    )
                wT = wtp.tile([P, 2 * CCH, P], BF16, tag="wT",
                              name=f"wT{pair}_{s}")
                nc.sync.dma_start_transpose(
                    out=wT, in_=wei[:].rearrange("p i u f -> p (i u f)")
                )
                d["wT"][pair] = wT

            def ph_att(s, j):
                d = st[s]
                ps_att = psa.tile([P, H, HD + 1], F32, tag="att",
                                  name=f"psatt{j}_{s}")
                for hh in range(H):
                    wT = d["wT"][hh // 2]
                    i = hh % 2
                    if j == 0:
                        nc.tensor.matmul(
                            ps_att[:, hh, :], wT[:, 3 * i, :],
                            d["v"][:, 0, hh, :],
                            start=True, stop=True,
                        )
                    else:
                        for cs in range(2):
                            nc.tensor.matmul(
                                ps_att[:, hh, :], wT[:, 3 * i + 1 + cs, :],
                                d["v"][:, cs, hh, :],
                                start=(cs == 0), stop=(cs == 1),
                            )
                rs = stp.tile([P, H], F32, tag="rs", name=f"rs{j}_{s}")
                nc.vector.reciprocal(out=rs, in_=ps_att[:, :, HD:HD + 1])
                if j == 0:
                    d["attsb"] = atp.tile([P, 2, C], BF16, tag="attsb",
                                          bufs=2, name=f"attsb_{s}")
                nc.vector.tensor_tensor(
                    out=d["attsb"][:, j, :].rearrange("p (h e) -> p h e", h=H),
                    in0=ps_att[:, :, 0:HD], in1=bcast(rs, HD), op=OP.mult,
                )
                if j == 1:
                    attT = atp.tile([P, 2 * CCH, P], BF16, tag="attT",
                                    bufs=2, name=f"attT_{s}")
                    nc.sync.dma_start_transpose(
                        out=attT, in_=d["attsb"][:].rearrange(
                            "p j f -> p (j f)")
                    )
                    d["attT"] = attT

            def ph_proj(s, j):
                d = st[s]
                ps_sa = psb.tile([P, C], F32, tag="big", name=f"ps_sa{j}_{s}")
                for c in range(CCH):
                    nc.tensor.matmul(
                        ps_sa, d["attT"][:, 3 * j + c, :], wp_sb[:, c, :],
                        start=(c == 0), stop=(c == CCH - 1 and not has_bp),
                    )
                if has_bp:
                    nc.tensor.matmul(ps_sa, ones_sb, bext_sb[:, 1, :],
                                     start=False, stop=True)
                if j == 0:
                    d["x2"] = xp.tile([P, 2, C], F32, tag="x2", bufs=2,
                                      name=f"x2_{s}")
                    d["h2"] = ap.tile([P, 2, C], BF16, tag="h2",
                                      name=f"h2_{s}")
                nc.vector.tensor_tensor(out=d["x2"][:, j, :], in0=ps_sa,
                                        in1=d["x"][:, j, :], op=OP.add)
                layer_norm(d["x2"][:, j, :], d["h2"][:, j, :], f"l2_{j}_{s}")
                if j == 1:
                    h2T = ap.tile([P, 2 * CCH, P], BF16, tag="h2T",
                                  name=f"h2T_{s}")
                    nc.sync.dma_start_transpose(
                        out=h2T, in_=d["h2"][:].rearrange("p j f -> p (j f)")
                    )
                    d["h2T"] = h2T

            def ph_ffn1(s, g2s):
                d = st[s]
                if "aT" not in d:
                    d["aT"] = ffp.tile([P, NFF, T], BF16, tag="aT",
                                       name=f"aT_{s}")
                h2r = d["h2T"][:].rearrange("p (j c) t -> p c j t", j=2)
                for g2 in g2s:
                    ps_a = psf.tile([P, 2, T], F32, tag="ff1",
                                    name=f"psa{g2}_{s}")
                    for i in range(2):
                        g = 2 * g2 + i
                        for c in range(CCH):
                            nc.tensor.matmul(
                                ps_a[:, i, :],
                                w1_sb[:, c, g * P:(g + 1) * P],
                                h2r[:, c, :, :],
                                start=(c == 0), stop=(c == CCH - 1),
                            )
                    if has_b1:
                        for i in range(2):
                            g = 2 * g2 + i
                            nc.scalar.activation(
                                out=d["aT"][:, g, :], in_=ps_a[:, i, :],
                                func=AF.Relu, bias=b1_sb[:, g:g + 1],
                                scale=1.0,
                            )
                    else:
                        nc.scalar.activation(
                            out=d["aT"][:, 2 * g2:2 * g2 + 2, :], in_=ps_a,
                            func=AF.Relu,
                        )

            def ph_ffn2(s, j):
                d = st[s]
                ps_y = psb.tile([P, C], F32, tag="big", name=f"ps_y{j}_{s}")
                for g in range(NFF):
                    nc.tensor.matmul(
                        ps_y, d["aT"][:, g, j * P:(j + 1) * P], w2_sb[:, g, :],
                        start=(g == 0), stop=(g == NFF - 1 and not has_b2),
                    )
                if has_b2:
                    nc.tensor.matmul(ps_y, ones_sb, bext_sb[:, 2, :],
                                     start=False, stop=True)
                if j == 0:
                    d["o"] = op_.tile([P, 2, C], F32, tag="o", name=f"o_{s}")
                nc.vector.tensor_tensor(out=d["o"][:, j, :], in0=ps_y,
                                        in1=d["x2"][:, j, :], op=OP.add)
                if j == 1:
                    it = 2 * s
                    nc.scalar.dma_start(
                        out=out_h[it * P:(it + 2) * P, :].rearrange(
                            "(j p) f -> p j f", p=P),
                        in_=d["o"],
                    )
                    st[s] = {}

            # ---- software-pipelined schedule ----
            # body i: attention+proj for seq i-1, S for seq i, LN1+QKV for
            # seq i+1, FFN for seq i-2.
            ph_x_dma(0)
            if B_SH > 1:
                ph_x_dma(1)
            ph_ln1(0)
            ph_v(0)
            ph_qk(0)
            for i in range(B_SH + 2):
                a = i - 1   # attention/proj seq
                f = i - 2   # ffn seq
                av = 0 <= a < B_SH
                fv = 0 <= f < B_SH
                if i + 1 < B_SH:
                    ph_ln1(i + 1)
                if av:
                    ph_att(a, 0)
                if i < B_SH:
                    ph_S(i, 0)
                if av:
                    ph_att(a, 1)
                if i < B_SH:
                    ph_S(i, 1)
                if fv:
                    ph_ffn1(f, [0, 1, 2])
                if i + 1 < B_SH:
                    ph_v(i + 1)
                if i < B_SH:
                    ph_S(i, 2)
                if i + 1 < B_SH:
                    ph_qk(i + 1)
                if av:
                    ph_proj(a, 0)
                    ph_proj(a, 1)
                if fv:
                    ph_ffn1(f, [3, 4, 5])
                    ph_ffn2(f, 0)
                    ph_ffn2(f, 1)
                if i + 2 < B_SH:
                    ph_x_dma(i + 2)

    _hoist_extra_waits(nc)
    return nc


def _prep_weights(inputs):
    f32 = np.float32
    g1 = inputs["ln1_g"].astype(f32)
    b1l = inputs["ln1_b"].astype(f32)
    g2 = inputs["ln2_g"].astype(f32)
    b2l = inputs["ln2_b"].astype(f32)
    wq, wk, wv = (inputs[k].astype(f32) for k in ("wq", "wk", "wv"))
    w1 = inputs["w1"].astype(f32)

    # fold LN gains/biases + attention scale
    wq_f = wq * g1[None, :, None] * SCALE          # [H, C, hd]
    bq = SCALE * np.einsum("c,hcd->hd", b1l, wq)   # [H, hd]
    wk_f = wk * g1[None, :, None]
    bk = np.einsum("c,hcd->hd", b1l, wk)
    wv_f = wv * g1[None, :, None]
    bv = np.einsum("c,hcd->hd", b1l, wv)
    w1_f = w1 * g2[:, None]
    b1f = inputs["b1"].astype(f32) + b2l @ w1

    # head-major column layout [C, H*hd]
    to_mat = lambda w: np.ascontiguousarray(w.transpose(1, 0, 2).reshape(C, C))
    tri = np.triu(np.full((P, P), -1e9, dtype=f32), k=1)
    d = {
        "wq_m": to_mat(wq_f).astype(_BF),
        "wk_m": to_mat(wk_f).astype(_BF),
        "wv_m": to_mat(wv_f).astype(_BF),
        "wp_m": np.ascontiguousarray(inputs["w_proj"].astype(f32)).astype(_BF),
        "w1_m": np.ascontiguousarray(w1_f).astype(_BF),
        "w2_m": np.ascontiguousarray(inputs["w2"].astype(f32)).astype(_BF),
        "b1_v": np.ascontiguousarray(b1f).astype(f32),
        "maskf_m": np.concatenate([tri, np.zeros((P, P), f32), tri], axis=1),
    }
    bv_r = bv.reshape(C)
    bp_r = inputs["b_proj"].astype(f32)
    b2_r = inputs["b2"].astype(f32)
    bq_r = bq.reshape(C)
    bk_r = bk.reshape(C)
    d["bext_v"] = np.stack([bv_r, bp_r, b2_r, bq_r, bk_r]).astype(_BF)
    flags = (bool(np.any(bv_r)), bool(np.any(bp_r)), bool(np.any(b2_r)),
             bool(np.any(b1f)),
             bool(np.any(bq_r)) or bool(np.any(bk_r)))
    return d, flags


def kernel(**inputs) -> np.ndarray:
    x = np.ascontiguousarray(inputs["x"].astype(np.float32))
    weights, flags = _prep_weights(inputs)

    if flags not in _CACHE:
        _CACHE[flags] = _build(*flags)
    nc = _CACHE[flags]

    xs = x.reshape(N_CORES, TOK, C)
    in_maps = [dict(weights, x=np.ascontiguousarray(xs[i])) for i in range(N_CORES)]
    import os

    kwargs = {}
    if os.environ.get("BASS_PROF"):
        kwargs = {"trace": True, "trace_cores": [0]}
    res = run_bass_kernel_spmd(nc, in_maps, list(range(N_CORES)), **kwargs)
    globals()["LAST_RESULTS"] = res
    out = np.stack([res.results[i]["out"] for i in range(N_CORES)])
    return out.reshape(B, T, C).astype(np.float32)
